# revision 1
# baseline (speedup 1.0000x reference)
"""Trainium2 Bass kernel for nn_DiffusionLayer_rec2_transformer (point-transformer
layer: KNN-16 attention over 8192 points, batch 2, 128 channels).

Self-contained: kernel(**inputs) -> np.ndarray [2, 128, 8192].

Distribution: 8 NeuronCores; core c handles batch c//4, query slice
(c%4)*2048. Each core receives its batch's full point set (column-rotated so
its own queries sit at columns 0..2047) and computes KNN + attention for its
2048 queries; GroupNorm statistics are combined across the 4 cores of each
batch with tiny AllReduces.

KNN exactness: coarse scores via an fp16-pair K=13 matmul (error ~1e-5),
per-512-chunk top-8 (DVE max8) + top-24 merge, then exact-fp32 refinement of
the 24 candidates from squared coordinate differences; verified to reproduce
jax fp32 top-16 sets exactly for this problem's data distribution.
"""
import dataclasses
import numpy as np
import concourse.bass as bass
import concourse.bacc as bacc
import concourse.tile as tile
from concourse import mybir
from concourse.bass_utils import run_bass_kernel_spmd

dt = mybir.dt
AF = mybir.ActivationFunctionType
ALU = mybir.AluOpType
AX = mybir.AxisListType

N = 8192
NQ = 2048
K = 16
CHUNK = 512
NCH = N // CHUNK          # 16 chunks
NCAND = 24
BIG = 1e30
F16BIG = 60000.0
EPS = 1e-5
NEG = 0.1


def build(n_cores=8, ntiles=16, groups=None, dbg=(), group_size=4):
    if groups is None:
        groups = [[0, 1, 2, 3], [4, 5, 6, 7]] if n_cores == 8 else [[c] for c in range(n_cores)]
    nc = bacc.Bacc("TRN2", target_bir_lowering=False, debug=False,
                   num_devices=n_cores)

    def din(name, shape, d=dt.float32):
        return nc.dram_tensor(name, shape, d, kind="ExternalInput")

    # ---- inputs (per-core, host-prepped; see npmodel.host_prep) ----
    feat = din("feat", [128, N])
    rhs13 = din("rhs13", [16, N], dt.float16)
    lhsT13 = din("lhsT13", [16, NQ], dt.float16)
    cT3 = din("cT3", [4, NQ])
    pcT = din("pcT", [NQ, 4])
    xyzrow = din("xyzrow", [4, N])
    wnames = ["LWpre", "LWq", "LWkneg", "LWv", "LWpos2a", "LWpos2b",
              "LWatt1", "LWatt2a", "LWatt2b", "LWpost"]
    W = {n_: din(n_, [128, 128]) for n_ in wnames}
    lhsT6 = din("lhsT6", [6, 128])
    bnames = ["Bpre", "Bv", "Battin", "Batt1", "Batt2", "Bpost", "Bpos1",
              "Gpos", "BEpos", "Gatt", "BEatt", "Gpost", "BEpost"]
    B = {n_: din(n_, [128, 1]) for n_ in bnames}
    ident = din("ident", [128, 128])
    mask384 = din("mask384", [128, 384])
    scat384 = din("scat384", [128, 384], dt.int16)
    scat256 = din("scat256", [128, 256], dt.int16)
    cT3r = din("cT3r", [4, NQ * K])
    blockones = din("blockones", [128, 8])      # BO[ch, g] = ch//16 == g
    blockonesT = din("blockonesT", [8, 128])

    out = nc.dram_tensor("out", [128, NQ], dt.float32, kind="ExternalOutput")
    dbg_t = {}
    for name, shape, d in (
        ("dbg_nf", [128, N], dt.float32),
        ("dbg_S", [128, N], dt.float16),
        ("dbg_M8", [128, 128], dt.float16),
        ("dbg_I8", [128, 128], dt.uint16),
        ("dbg_P24", [128, 24], dt.uint16),
        ("dbg_gidx24", [128, 24], dt.uint32),
        ("dbg_gidx16", [128, 16], dt.uint32),
        ("dbg_pos1", [128, NQ], dt.float32),
        ("dbg_stats", [128, 8], dt.float32),
        ("dbg_poshid", [128, NQ], dt.float32),
        ("dbg_attin", [128, NQ], dt.float32),
        ("dbg_att1", [128, NQ], dt.float32),
        ("dbg_e", [128, NQ], dt.float32),
        ("dbg_vg", [128, NQ], dt.float32),
        ("dbg_out1", [128, ntiles * 128], dt.float32),
        ("dbg_post", [128, ntiles * 128], dt.float32),
    ):
        if name in dbg:
            dbg_t[name] = nc.dram_tensor(name, shape, d, kind="ExternalOutput")

    # internal DRAM
    pos1_spill = nc.dram_tensor("pos1_spill", [128, ntiles * NQ], dt.float32)
    att1_spill = nc.dram_tensor("att1_spill", [128, ntiles * NQ], dt.float32)
    out1_spill = nc.dram_tensor("out1_spill", [128, ntiles * 128], dt.float32)
    cc = [(nc.dram_tensor(f"cc{i}_in", [128, 2], dt.float32),
           nc.dram_tensor(f"cc{i}_out", [128, 2], dt.float32)) for i in range(3)]

    COLS = ntiles * NQ * group_size          # spatial cols per batch (N*K when full)
    M_big = 16 * COLS                        # gnorm count (pos/att)
    M_post = 16 * ntiles * 128 * group_size  # gnorm count (post)

    import dataclasses

    def diag_extract(dst, src, nsel):
        """dst[q, c] = src[q, 16*c + q%16]; src [128, 16*nsel], dst [128, nsel]."""
        F = 16 * nsel
        for p8 in range(8):
            sap = dataclasses.replace(src[:], ap=[[F + 16, 16], [16, nsel]],
                                      offset=src[:].offset + p8 * 16 * F)
            dap = dataclasses.replace(dst[:], ap=[[nsel, 16], [1, nsel]],
                                      offset=dst[:].offset + p8 * 16 * nsel)
            nc.sync.dma_start(dap, sap)

    with tile.TileContext(nc) as tc:
        with (
            tc.tile_pool(name="pers", bufs=1) as pers,
            tc.tile_pool(name="work", bufs=1) as work,
            tc.tile_pool(name="big", bufs=4) as bigp,
            tc.tile_pool(name="psA", bufs=2, space="PSUM") as psA,
            tc.tile_pool(name="psB", bufs=2, space="PSUM") as psB,
            tc.tile_pool(name="psT", bufs=1, space="PSUM") as psT,
        ):
            f32, f16, u16, u32, i16 = dt.float32, dt.float16, dt.uint16, dt.uint32, dt.int16

            # ---------- persistent tiles ----------
            NF = pers.tile([128, N], f32, name="NF")
            XR = [pers.tile([128, N], f32, name=f"XR{c}") for c in range(3)]
            WRG = pers.tile([128, ntiles * 128], i16, name="WRG")
            Wt = {n_: pers.tile([128, 128], f32, name="t" + n_) for n_ in wnames}
            L16 = pers.tile([6, 128], f32, name="L16")
            Bt = {n_: pers.tile([128, 1], f32, name="t" + n_) for n_ in bnames}
            IDENT = pers.tile([128, 128], f32, name="IDENT")
            BO = pers.tile([128, 8], f32, name="BO")
            BOT = pers.tile([8, 128], f32, name="BOT")
            CB = pers.tile([128, 128], u32, name="CB")      # chunk base iota
            M0 = pers.tile([128, 384], f32, name="M0")      # refine mask
            PC = pers.tile([NQ, 4], f32, name="PCfull") if False else None
            STAT = pers.tile([128, 2], f32, name="STAT")    # running sums (pos)
            STAT2 = pers.tile([128, 2], f32, name="STAT2")  # (att)
            STAT3 = pers.tile([128, 2], f32, name="STAT3")  # (post)
            EPST = pers.tile([8, 1], f32, name="EPST")

            # ---------- load constants ----------
            for n_ in wnames:
                nc.sync.dma_start(Wt[n_][:], W[n_].ap())
            for n_ in bnames:
                nc.sync.dma_start(Bt[n_][:], B[n_].ap())
            nc.sync.dma_start(L16[:], lhsT6.ap())
            nc.sync.dma_start(IDENT[:], ident.ap())
            nc.sync.dma_start(BO[:], blockones.ap())
            nc.sync.dma_start(BOT[:], blockonesT.ap())
            nc.gpsimd.iota(CB[:], pattern=[[512, 16], [0, 8]], base=0,
                           channel_multiplier=0)
            nc.sync.dma_start(M0[:], mask384.ap())
            SC384 = pers.tile([128, 384], i16, name="SC384")
            SC256 = pers.tile([128, 256], i16, name="SC256")
            nc.sync.dma_start(SC384[:], scat384.ap())
            nc.sync.dma_start(SC256[:], scat256.ap())
            nc.gpsimd.memset(STAT[:], 0.0)
            nc.gpsimd.memset(STAT2[:], 0.0)
            nc.gpsimd.memset(STAT3[:], 0.0)
            nc.gpsimd.memset(EPST[:], EPS)

            # xyz replicated across partitions (doubling broadcast)
            for c in range(3):
                nc.sync.dma_start(XR[c][0:1, :], xyzrow.ap()[c:c + 1, :])
                p = 1
                while p < 128:
                    nc.sync.dma_start(XR[c][p:2 * p, :], XR[c][0:p, :])
                    p *= 2

            # ---------- phase 0: nf ----------
            for c in range(NCH):
                FC = work.tile([128, 512], f32, tag="FCSc", bufs=2, name="FC")
                nc.sync.dma_start(FC[:], feat.ap()[:, c * 512:(c + 1) * 512])
                pb = psA.tile([128, 512], f32, tag="pA", name="pnf")
                nc.tensor.matmul(pb[:], Wt["LWpre"][:], FC[:])
                nc.scalar.activation(NF[:, c * 512:(c + 1) * 512], pb[:],
                                     AF.Identity, bias=Bt["Bpre"][:])
            if "dbg_nf" in dbg_t:
                nc.sync.dma_start(dbg_t["dbg_nf"].ap(), NF[:])

            # ================= PHASE A (per tile) =================
            for t in range(ntiles):
                toff = t * 128
                M8 = work.tile([128, 128], f16, tag="M8", name="M8")
                I8 = work.tile([128, 128], u16, tag="I8", name="I8")
                L13t = work.tile([16, 128], f16, tag="L13t", bufs=2, name="L13t")
                nc.sync.dma_start(L13t[:], lhsT13.ap()[:, toff:toff + 128])
                for c in range(NCH):
                    R13c = work.tile([16, 512], f16, tag="R13c", bufs=2, name="R13c")
                    nc.sync.dma_start(R13c[:], rhs13.ap()[:, c * 512:(c + 1) * 512])
                    pb = psA.tile([128, 512], f32, tag="pA", name="pdist")
                    nc.tensor.matmul(pb[:], L13t[:], R13c[:])
                    Sc = work.tile([128, 512], f16, tag="Sc", bufs=2, name="Sc")
                    nc.scalar.copy(Sc[:], pb[:])
                    nc.vector.max(M8[:, 8 * c:8 * c + 8], Sc[:])
                    nc.vector.max_index(I8[:, 8 * c:8 * c + 8],
                                        M8[:, 8 * c:8 * c + 8], Sc[:])
                    if "dbg_S" in dbg_t and t == 0:
                        nc.sync.dma_start(dbg_t["dbg_S"].ap()[:, c * 512:(c + 1) * 512], Sc[:])
                if "dbg_M8" in dbg_t and t == 0:
                    nc.sync.dma_start(dbg_t["dbg_M8"].ap(), M8[:])
                if "dbg_I8" in dbg_t and t == 0:
                    nc.sync.dma_start(dbg_t["dbg_I8"].ap(), I8[:])

                # Iglob = u32(I8) + 512*chunk
                IG = work.tile([128, 128], u32, tag="IG", name="IG")
                nc.vector.tensor_copy(IG[:], I8[:])
                nc.vector.tensor_tensor(IG[:], IG[:], CB[:], ALU.add)

                # stage 2: top-24 positions of M8
                P24 = work.tile([128, 24], u16, tag="P24", name="P24")
                W8 = work.tile([128, 8], f16, tag="W8", name="W8")
                for r in range(3):
                    nc.vector.max(W8[:], M8[:])
                    nc.vector.max_index(P24[:, 8 * r:8 * r + 8], W8[:], M8[:])
                    if r < 2:
                        nc.vector.match_replace(M8[:], W8[:], M8[:], -F16BIG)
                if "dbg_P24" in dbg_t and t == 0:
                    nc.sync.dma_start(dbg_t["dbg_P24"].ap(), P24[:])

                # gather Iglob at P24 (per-core lists) -> diag extract gidx24
                G384 = work.tile([128, 384], u32, tag="G384", name="G384")
                nc.gpsimd.ap_gather(
                    G384[:], IG[:].rearrange("p (f o) -> p f o", o=1),
                    P24[:].bitcast(i16), channels=128, num_elems=128, d=1,
                    num_idxs=384)
                G384h = work.tile([128, 384], u16, tag="G384h", name="G384h")
                nc.vector.tensor_copy(G384h[:], G384[:])
                GI24w = work.tile([128, 24], u16, tag="GI24w", name="GI24w")
                nc.gpsimd.local_scatter(GI24w[:], G384h[:], SC384[:],
                                        channels=128, num_elems=24, num_idxs=384)
                GI24 = work.tile([128, 24], u32, tag="GI24", name="GI24")
                nc.vector.tensor_copy(GI24[:], GI24w[:])
                if "dbg_gidx24" in dbg_t and t == 0:
                    nc.sync.dma_start(dbg_t["dbg_gidx24"].ap(), GI24[:])

                # refine: gather xyz at candidates, exact d2
                GX = [work.tile([128, 384], f32, tag=f"GX{c}", name=f"GX{c}")
                      for c in range(3)]
                for c in range(3):
                    nc.gpsimd.ap_gather(
                        GX[c][:], XR[c][:].rearrange("p (f o) -> p f o", o=1),
                        GI24w[:].bitcast(i16), channels=128, num_elems=N, d=1,
                        num_idxs=384)
                PCt = work.tile([128, 4], f32, tag="PCt", name="PCt")
                nc.sync.dma_start(PCt[:], pcT.ap()[toff:toff + 128, :])
                SNM = work.tile([128, 384], f32, tag="SNM", name="SNM")
                SQ1 = work.tile([128, 384], f32, tag="SQS", name="SQ1")
                for c in range(3):
                    d_ = GX[c]
                    nc.vector.tensor_tensor(
                        d_[:], d_[:], PCt[:, c:c + 1].broadcast_to([128, 384]),
                        ALU.subtract)
                nc.scalar.activation(SNM[:], GX[0][:], AF.Square)
                nc.scalar.activation(SQ1[:], GX[1][:], AF.Square)
                nc.vector.tensor_tensor(SNM[:], SNM[:], SQ1[:], ALU.add)
                nc.scalar.activation(SQ1[:], GX[2][:], AF.Square)
                nc.vector.tensor_tensor(SNM[:], SNM[:], SQ1[:], ALU.add)
                # snm = M0 - d2  (own positions: -d2; others: -BIG)
                nc.vector.tensor_tensor(SNM[:], M0[:], SNM[:], ALU.subtract)

                P16 = work.tile([128, 16], u16, tag="P16", name="P16")
                W8f = work.tile([128, 8], f32, tag="W8f", name="W8f")
                for r in range(2):
                    nc.vector.max(W8f[:], SNM[:])
                    nc.vector.max_index(P16[:, 8 * r:8 * r + 8], W8f[:], SNM[:])
                    if r < 1:
                        nc.vector.match_replace(SNM[:], W8f[:], SNM[:], -BIG)
                # c16 = P16 >> 4 (position -> candidate rank)
                C16 = work.tile([128, 16], u16, tag="C16", name="C16")
                nc.vector.tensor_scalar(C16[:], P16[:], 4, None,
                                        ALU.logical_shift_right)
                G256 = work.tile([128, 256], u32, tag="G256", name="G256")
                nc.gpsimd.ap_gather(
                    G256[:], GI24[:].rearrange("p (f o) -> p f o", o=1),
                    C16[:].bitcast(i16), channels=128, num_elems=24, d=1,
                    num_idxs=256)
                G256h = work.tile([128, 256], u16, tag="G256h", name="G256h")
                nc.vector.tensor_copy(G256h[:], G256[:])
                GI16w = work.tile([128, 16], u16, tag="GI16w", name="GI16w")
                nc.gpsimd.local_scatter(GI16w[:], G256h[:], SC256[:],
                                        channels=128, num_elems=16, num_idxs=256)
                GI16 = work.tile([128, 16], u32, tag="GI16", name="GI16")
                nc.vector.tensor_copy(GI16[:], GI16w[:])
                if "dbg_gidx16" in dbg_t and t == 0:
                    nc.sync.dma_start(dbg_t["dbg_gidx16"].ap(), GI16[:])

                # wrg slot: transpose(gidx16) replicated x8
                GI16f = work.tile([128, 16], f32, tag="GI16f", name="GI16f")
                nc.vector.tensor_copy(GI16f[:], GI16[:])
                ptr = psT.tile([16, 128], f32, tag="ptr", name="ptr")
                nc.tensor.transpose(ptr[:], GI16f[:], IDENT[:])
                TGf = work.tile([16, 128], f32, tag="TGf", name="TGf")
                nc.scalar.copy(TGf[:], ptr[:])
                wslot = WRG[:, t * 128:(t + 1) * 128]
                nc.vector.tensor_copy(wslot[0:16, :], TGf[:])
                p = 16
                while p < 128:
                    nc.sync.dma_start(wslot[p:2 * p, :], wslot[0:p, :])
                    p *= 2

                # pos1: rhs16 = [xyzg(3); 0; centers(3); 0...]
                PP = bigp.tile([128, NQ], f32, tag="big", name="PP")
                SQS = work.tile([128, 512], f32, tag="SQS", name="SQS")
                A1 = work.tile([128, 1], f32, tag="A1", name="A1")
                A2 = work.tile([128, 1], f32, tag="A2", name="A2")
                for u in range(4):
                    R6 = work.tile([6, 512], f32, tag="R6", bufs=2, name="R6")
                    for c in range(3):
                        XGc = work.tile([16, 512], f32, tag="XGc", bufs=2, name="XGc")
                        nc.gpsimd.ap_gather(
                            XGc[:], XR[c][0:16, :].rearrange("p (f o) -> p f o", o=1),
                            wslot[0:16, 32 * u:32 * u + 32].bitcast(i16),
                            channels=16, num_elems=N, d=1, num_idxs=512)
                        nc.sync.dma_start(R6[c:c + 1, :], XGc[0:1, :])
                    nc.sync.dma_start(
                        R6[3:6, :],
                        cT3r.ap()[0:3, toff * 16 + 512 * u:toff * 16 + 512 * (u + 1)])
                    pb = psB.tile([128, 512], f32, tag="pB", name="ppos1")
                    nc.tensor.matmul(pb[:], L16[:], R6[:])
                    sl = PP[:, u * 512:(u + 1) * 512]
                    nc.scalar.activation(sl, pb[:], AF.Identity,
                                         bias=Bt["Bpos1"][:], accum_out=A1[:])
                    nc.scalar.activation(SQS[:], sl, AF.Square, accum_out=A2[:])
                    nc.vector.tensor_tensor(STAT[:, 0:1], STAT[:, 0:1], A1[:], ALU.add)
                    nc.vector.tensor_tensor(STAT[:, 1:2], STAT[:, 1:2], A2[:], ALU.add)
                nc.sync.dma_start(pos1_spill.ap()[:, t * NQ:(t + 1) * NQ], PP[:])
                if "dbg_pos1" in dbg_t and t == 0:
                    nc.sync.dma_start(dbg_t["dbg_pos1"].ap(), PP[:])

            # ---------- allreduce pos stats + scale/bias ----------
            def allreduce_stats(stat, ccpair, Mcount, Gt, BEt, tag):
                ccin, ccout = ccpair
                nc.sync.dma_start(ccin.ap(), stat[:])
                nc.gpsimd.collective_compute(
                    "AllReduce", ALU.add, replica_groups=groups,
                    ins=[ccin.ap().opt()], outs=[ccout.ap().opt()])
                ST = work.tile([128, 2], f32, tag="ST" + tag, name="ST" + tag)
                nc.sync.dma_start(ST[:], ccout.ap())
                pg = psT.tile([8, 2], f32, tag="pg", name="pg" + tag)
                nc.tensor.matmul(pg[:], BO[:], ST[:])
                GS = work.tile([8, 2], f32, tag="GS" + tag, name="GS" + tag)
                nc.scalar.copy(GS[:], pg[:])
                MM = work.tile([8, 4], f32, tag="MM" + tag, name="MM" + tag)
                nc.vector.tensor_scalar(MM[:, 0:1], GS[:, 0:1], 1.0 / Mcount, None, ALU.mult)
                nc.vector.tensor_scalar(MM[:, 1:2], GS[:, 1:2], 1.0 / Mcount, None, ALU.mult)
                nc.vector.tensor_tensor(MM[:, 2:3], MM[:, 0:1], MM[:, 0:1], ALU.mult)
                nc.vector.tensor_tensor(MM[:, 2:3], MM[:, 1:2], MM[:, 2:3], ALU.subtract)
                # rs = 1/sqrt(var+eps)
                nc.scalar.activation(MM[:, 3:4], MM[:, 2:3], AF.Sqrt, bias=EPST[:])
                nc.vector.reciprocal(MM[:, 3:4], MM[:, 3:4])
                # broadcast to [128,1]
                pr = psT.tile([128, 2], f32, tag="pr", name="pr" + tag)
                nc.tensor.matmul(pr[:, 0:1], BOT[:], MM[:, 3:4])
                nc.tensor.matmul(pr[:, 1:2], BOT[:], MM[:, 0:1])
                SCB = work.tile([128, 2], f32, tag="SCB" + tag, name="SCB" + tag)
                nc.scalar.copy(SCB[:], pr[:])
                SC = work.tile([128, 1], f32, tag="SC" + tag, name="SC" + tag)
                BI = work.tile([128, 1], f32, tag="BI" + tag, name="BI" + tag)
                nc.vector.tensor_tensor(SC[:], SCB[:, 0:1], Gt[:], ALU.mult)
                nc.vector.tensor_tensor(BI[:], SCB[:, 1:2], SC[:], ALU.mult)
                nc.vector.tensor_tensor(BI[:], BEt[:], BI[:], ALU.subtract)
                return SC, BI

            SCp, BIp = allreduce_stats(STAT, cc[0], M_big, Bt["Gpos"], Bt["BEpos"], "p")

            # ================= PHASE B (per tile) =================
            for t in range(ntiles):
                PL = bigp.tile([128, NQ], f32, tag="big", name="PL")
                nc.sync.dma_start(PL[:], pos1_spill.ap()[:, t * NQ:(t + 1) * NQ])
                ZH = bigp.tile([128, NQ], f32, tag="big", name="ZH")
                ZA = bigp.tile([128, NQ], f32, tag="big", name="ZA")
                nc.scalar.activation(ZH[:], PL[:], AF.Identity, bias=BIp[:], scale=SCp[:])
                nc.scalar.activation(ZA[:], PL[:], AF.Abs, bias=BIp[:], scale=SCp[:])
                if "dbg_poshid" in dbg_t and t == 0:
                    nc.sync.dma_start(dbg_t["dbg_poshid"].ap(), ZH[:])
                NFG = bigp.tile([128, NQ], f32, tag="big", name="NFG")
                wslot = WRG[:, t * 128:(t + 1) * 128]
                nc.gpsimd.ap_gather(
                    NFG[:], NF[:].rearrange("p (f o) -> p f o", o=1),
                    wslot.bitcast(i16), channels=128, num_elems=N, d=1, num_idxs=NQ)
                AT = bigp.tile([128, NQ], f32, tag="big", name="AT")
                A1T = bigp.tile([128, NQ], f32, tag="big", name="A1T")
                SQS = work.tile([128, 512], f32, tag="SQS", name="SQSb")
                A1 = work.tile([128, 1], f32, tag="A1", name="A1b")
                A2 = work.tile([128, 1], f32, tag="A2", name="A2b")
                for c in range(4):
                    pb = psB.tile([128, 512], f32, tag="pB", name="pattin")
                    qof = t * 128 + c * 32
                    nc.tensor.matmul(
                        pb[:], Wt["LWq"][:],
                        NF[:, qof:qof + 32].rearrange("p (q o) -> p q o", o=1)
                        .broadcast_to([128, 32, 16]), start=True, stop=False)
                    nc.tensor.matmul(pb[:], Wt["LWkneg"][:],
                                     NFG[:, c * 512:(c + 1) * 512],
                                     start=False, stop=False)
                    nc.tensor.matmul(pb[:], Wt["LWpos2a"][:],
                                     ZH[:, c * 512:(c + 1) * 512],
                                     start=False, stop=False)
                    nc.tensor.matmul(pb[:], Wt["LWpos2b"][:],
                                     ZA[:, c * 512:(c + 1) * 512],
                                     start=False, stop=True)
                    nc.scalar.activation(AT[:, c * 512:(c + 1) * 512], pb[:],
                                         AF.Identity, bias=Bt["Battin"][:])
                    pb2 = psA.tile([128, 512], f32, tag="pA", name="patt1")
                    nc.tensor.matmul(pb2[:], Wt["LWatt1"][:],
                                     AT[:, c * 512:(c + 1) * 512])
                    sl = A1T[:, c * 512:(c + 1) * 512]
                    nc.scalar.activation(sl, pb2[:], AF.Identity,
                                         bias=Bt["Batt1"][:], accum_out=A1[:])
                    nc.scalar.activation(SQS[:], sl, AF.Square, accum_out=A2[:])
                    nc.vector.tensor_tensor(STAT2[:, 0:1], STAT2[:, 0:1], A1[:], ALU.add)
                    nc.vector.tensor_tensor(STAT2[:, 1:2], STAT2[:, 1:2], A2[:], ALU.add)
                nc.sync.dma_start(att1_spill.ap()[:, t * NQ:(t + 1) * NQ], A1T[:])
                if "dbg_attin" in dbg_t and t == 0:
                    nc.sync.dma_start(dbg_t["dbg_attin"].ap(), AT[:])
                if "dbg_att1" in dbg_t and t == 0:
                    nc.sync.dma_start(dbg_t["dbg_att1"].ap(), A1T[:])

            SCa, BIa = allreduce_stats(STAT2, cc[1], M_big, Bt["Gatt"], Bt["BEatt"], "a")

            # ================= PHASE C (per tile) =================
            for t in range(ntiles):
                AL = bigp.tile([128, NQ], f32, tag="big", name="AL")
                nc.sync.dma_start(AL[:], att1_spill.ap()[:, t * NQ:(t + 1) * NQ])
                AFt = bigp.tile([128, NQ], f32, tag="big", name="AFt")
                AFa = bigp.tile([128, NQ], f32, tag="big", name="AFa")
                nc.scalar.activation(AFt[:], AL[:], AF.Identity, bias=BIa[:], scale=SCa[:])
                nc.scalar.activation(AFa[:], AL[:], AF.Abs, bias=BIa[:], scale=SCa[:])
                NFG = bigp.tile([128, NQ], f32, tag="big", name="NFGc")
                wslot = WRG[:, t * 128:(t + 1) * 128]
                nc.gpsimd.ap_gather(
                    NFG[:], NF[:].rearrange("p (f o) -> p f o", o=1),
                    wslot.bitcast(i16), channels=128, num_elems=N, d=1, num_idxs=NQ)
                E = bigp.tile([128, NQ], f32, tag="big", name="E")
                VG = bigp.tile([128, NQ], f32, tag="big", name="VG")
                for c in range(4):
                    pb = psB.tile([128, 512], f32, tag="pB", name="patt2")
                    nc.tensor.matmul(pb[:], Wt["LWatt2a"][:],
                                     AFt[:, c * 512:(c + 1) * 512],
                                     start=True, stop=False)
                    nc.tensor.matmul(pb[:], Wt["LWatt2b"][:],
                                     AFa[:, c * 512:(c + 1) * 512],
                                     start=False, stop=True)
                    nc.scalar.activation(E[:, c * 512:(c + 1) * 512], pb[:],
                                         AF.Exp, bias=Bt["Batt2"][:])
                    pb2 = psA.tile([128, 512], f32, tag="pA", name="pvg")
                    nc.tensor.matmul(pb2[:], Wt["LWv"][:],
                                     NFG[:, c * 512:(c + 1) * 512])
                    nc.scalar.activation(VG[:, c * 512:(c + 1) * 512], pb2[:],
                                         AF.Identity, bias=Bt["Bv"][:])
                if "dbg_e" in dbg_t and t == 0:
                    nc.sync.dma_start(dbg_t["dbg_e"].ap(), E[:])
                if "dbg_vg" in dbg_t and t == 0:
                    nc.sync.dma_start(dbg_t["dbg_vg"].ap(), VG[:])
                SE = work.tile([128, 128], f32, tag="SE", name="SE")
                WS = work.tile([128, 128], f32, tag="WS", name="WS")
                EV = bigp.tile([128, NQ], f32, tag="big", name="EV")
                nc.vector.tensor_reduce(SE[:], E[:].rearrange("p (q j) -> p q j", j=16),
                                        axis=AX.X, op=ALU.add)
                nc.vector.tensor_tensor(EV[:], E[:], VG[:], ALU.mult)
                nc.vector.tensor_reduce(WS[:], EV[:].rearrange("p (q j) -> p q j", j=16),
                                        axis=AX.X, op=ALU.add)
                nc.vector.reciprocal(SE[:], SE[:])
                nc.vector.tensor_tensor(WS[:], WS[:], SE[:], ALU.mult)
                O1t = work.tile([128, 128], f32, tag="O1t", bufs=2, name="O1t")
                nc.vector.tensor_tensor(O1t[:], WS[:],
                                        NF[:, t * 128:(t + 1) * 128], ALU.add)
                nc.sync.dma_start(out1_spill.ap()[:, t * 128:(t + 1) * 128], O1t[:])


            # ---------- post conv + stats ----------
            PST = bigp.tile([128, ntiles * 128], f32, tag="big", name="PST")
            SQS = work.tile([128, 512], f32, tag="SQS", name="SQSp")
            A1 = work.tile([128, 1], f32, tag="A1", name="A1p")
            A2 = work.tile([128, 1], f32, tag="A2", name="A2p")
            npost = ntiles * 128
            for c in range((npost + 511) // 512):
                w = min(512, npost - c * 512)
                OC = work.tile([128, 512], f32, tag="FCSc", bufs=2, name="OC")
                nc.sync.dma_start(OC[:, :w], out1_spill.ap()[:, c * 512:c * 512 + w])
                pb = psB.tile([128, 512], f32, tag="pB", name="ppost")
                nc.tensor.matmul(pb[:, :w], Wt["LWpost"][:], OC[:, :w])
                sl = PST[:, c * 512:c * 512 + w]
                nc.scalar.activation(sl, pb[:, :w], AF.Identity,
                                     bias=Bt["Bpost"][:], accum_out=A1[:])
                nc.scalar.activation(SQS[:, :w], sl, AF.Square, accum_out=A2[:])
                nc.vector.tensor_tensor(STAT3[:, 0:1], STAT3[:, 0:1], A1[:], ALU.add)
                nc.vector.tensor_tensor(STAT3[:, 1:2], STAT3[:, 1:2], A2[:], ALU.add)
            if "dbg_post" in dbg_t:
                nc.sync.dma_start(dbg_t["dbg_post"].ap(), PST[:])

            SCq, BIq = allreduce_stats(STAT3, cc[2], M_post, Bt["Gpost"], Bt["BEpost"], "q")

            # ---------- final: leaky(norm(post)) ----------
            FZ = bigp.tile([128, ntiles * 128], f32, tag="big", name="FZ")
            FA = bigp.tile([128, ntiles * 128], f32, tag="big", name="FA")
            SC055 = work.tile([128, 1], f32, tag="SC055", name="SC055")
            BI055 = work.tile([128, 1], f32, tag="BI055", name="BI055")
            SC045 = work.tile([128, 1], f32, tag="SC045", name="SC045")
            BI045 = work.tile([128, 1], f32, tag="BI045", name="BI045")
            h1, h2 = (1 + NEG) / 2, (1 - NEG) / 2
            nc.vector.tensor_scalar(SC055[:], SCq[:], h1, None, ALU.mult)
            nc.vector.tensor_scalar(BI055[:], BIq[:], h1, None, ALU.mult)
            nc.vector.tensor_scalar(SC045[:], SCq[:], h2, None, ALU.mult)
            nc.vector.tensor_scalar(BI045[:], BIq[:], h2, None, ALU.mult)
            nc.scalar.activation(FZ[:], PST[:], AF.Identity, bias=BI055[:], scale=SC055[:])
            nc.scalar.activation(FA[:], PST[:], AF.Abs, bias=BI045[:], scale=SC045[:])
            nc.vector.tensor_tensor(FZ[:], FZ[:], FA[:], ALU.add)
            nc.sync.dma_start(out.ap()[:, 0:npost], FZ[:])

    nc.compile()
    return nc


NEG_SLOPE = 0.1


def _host_prep(xyz_b, feat_b, W):
    """Per-core inputs from (already rotated) xyz [3,N], feat [128,N]."""
    pts = xyz_b.T.astype(np.float32)
    sq = (pts * pts).sum(-1).astype(np.float32)
    u = (2.0 * pts).astype(np.float32)
    uhi = u.astype(np.float16)
    ulo = (u - uhi.astype(np.float32)).astype(np.float16)
    phi = pts.astype(np.float16)
    plo = (pts - phi.astype(np.float32)).astype(np.float16)
    shi = sq.astype(np.float16)
    slo = (sq - shi.astype(np.float32)).astype(np.float16)

    rhs13 = np.zeros((16, N), np.float16)
    rhs13[0:3] = phi.T
    rhs13[3:6] = plo.T
    rhs13[6:9] = phi.T
    rhs13[9] = -np.ones(N, np.float16)
    rhs13[10] = -np.ones(N, np.float16)
    rhs13[11] = -shi
    rhs13[12] = -slo

    qsl = slice(0, NQ)
    lhsT13 = np.zeros((16, NQ), np.float16)
    lhsT13[0:3] = uhi[qsl].T
    lhsT13[3:6] = uhi[qsl].T
    lhsT13[6:9] = ulo[qsl].T
    lhsT13[9] = shi[qsl]
    lhsT13[10] = slo[qsl]
    lhsT13[11] = np.ones(NQ, np.float16)
    lhsT13[12] = np.ones(NQ, np.float16)

    cT3 = np.zeros((4, NQ), np.float32)
    cT3[0:3] = xyz_b[:, qsl]
    pc = np.zeros((NQ, 4), np.float32)
    pc[:, 0:3] = pts[qsl]
    xyzrow = np.zeros((4, N), np.float32)
    xyzrow[0:3] = xyz_b
    cT3r = np.zeros((4, NQ * K), np.float32)
    cT3r[0:3] = np.repeat(cT3[0:3], K, axis=1)
    mask384 = np.full((128, 384), -BIG, np.float32)
    scat384 = np.full((128, 384), -1, np.int16)
    scat256 = np.full((128, 256), -1, np.int16)
    for q in range(128):
        mask384[q, q % 16::16] = 0.0
        scat384[q, q % 16::16] = np.arange(24, dtype=np.int16)
        scat256[q, q % 16::16] = np.arange(16, dtype=np.int16)
    lt = lambda m: np.ascontiguousarray(m.T)
    h1, h2 = (1 + NEG_SLOPE) / 2, (1 - NEG_SLOPE) / 2
    bo = np.zeros((128, 8), np.float32)
    for ch in range(128):
        bo[ch, ch // 16] = 1.0
    d = {
        "feat": feat_b.astype(np.float32),
        "rhs13": rhs13, "lhsT13": lhsT13, "cT3": cT3, "pcT": pc,
        "xyzrow": xyzrow, "cT3r": cT3r, "mask384": mask384,
        "scat384": scat384, "scat256": scat256,
        "LWpre": lt(W["W_pre"]), "LWq": lt(W["W_q"]),
        "LWkneg": lt(-W["W_k"]), "LWv": lt(W["W_v"]),
        "LWpos2a": lt(W["W_pos2"]) * h1, "LWpos2b": lt(W["W_pos2"]) * h2,
        "LWatt1": lt(W["W_att1"]),
        "LWatt2a": lt(W["W_att2"]) * h1, "LWatt2b": lt(W["W_att2"]) * h2,
        "LWpost": lt(W["W_post"]),
        "lhsT6": np.concatenate([W["W_pos1"].T, -W["W_pos1"].T]).astype(np.float32),
        "Bpre": W["b_pre"].reshape(128, 1),
        "Bv": W["b_v"].reshape(128, 1),
        "Battin": (W["b_q"] - W["b_k"] + W["b_pos2"]).reshape(128, 1),
        "Batt1": W["b_att1"].reshape(128, 1),
        "Batt2": W["b_att2"].reshape(128, 1),
        "Bpost": W["b_post"].reshape(128, 1),
        "Bpos1": W["b_pos1"].reshape(128, 1),
        "Gpos": W["g_pos1"].reshape(128, 1),
        "BEpos": W["be_pos1"].reshape(128, 1),
        "Gatt": W["g_att1"].reshape(128, 1),
        "BEatt": W["be_att1"].reshape(128, 1),
        "Gpost": W["g_post"].reshape(128, 1),
        "BEpost": W["be_post"].reshape(128, 1),
        "ident": np.eye(128, dtype=np.float32),
        "blockones": bo,
        "blockonesT": np.ascontiguousarray(bo.T),
    }
    return {k: np.ascontiguousarray(np.asarray(v, dtype=v.dtype if hasattr(v, "dtype") else np.float32)) for k, v in d.items()}


_CACHE = {}


def kernel(**inputs) -> np.ndarray:
    xyz = np.asarray(inputs["xyz"], np.float32)    # [2, 3, 8192]
    feat = np.asarray(inputs["feat"], np.float32)  # [2, 128, 8192]
    W = {k: np.asarray(v, np.float32) for k, v in inputs.items()
         if k not in ("xyz", "feat")}

    if "nc" not in _CACHE:
        _CACHE["nc"] = build(n_cores=8, ntiles=16)
    nc = _CACHE["nc"]

    in_maps = []
    for c in range(8):
        b, qs = c // 4, (c % 4) * NQ
        rot = np.roll(np.arange(N), -qs)
        in_maps.append(_host_prep(xyz[b][:, rot], feat[b][:, rot], W))

    res = run_bass_kernel_spmd(nc, in_maps, list(range(8)))
    outp = np.zeros((2, 128, N), np.float32)
    for c in range(8):
        b, qs = c // 4, (c % 4) * NQ
        outp[b][:, qs:qs + NQ] = res.results[c]["out"]
    return outp



# revision 24
# speedup vs baseline: 25.1067x; 25.1067x over previous
"""Trainium2 Bass kernel for nn_DiffusionLayer_rec2_transformer (point-transformer
layer: KNN-16 attention over 8192 points, batch 2, 128 channels).

Self-contained: kernel(**inputs) -> np.ndarray [2, 128, 8192].

Distribution: 8 NeuronCores; core c handles batch c//4, query slice
(c%4)*2048 (global, unrotated layout). Each core uploads ONLY its query
slice of feat (f16) + small xyz-derived per-slice tensors; full-point-set
tensors (pre-conv features NF, xyz rows, fp16 distance rhs) are rebuilt
on device with AllGathers inside each batch's 4-core group. GroupNorm
statistics are combined with tiny AllReduces. Weight-derived device arrays
are cached across calls (uploaded once per weight set); the jitted PJRT
executable is cached so per-call host work is concat + upload of ~700KiB
per core.

KNN exactness: coarse scores via an fp16-pair K=13 matmul, per-512-chunk
top-8 (DVE max8) + top-24 merge, then exact-fp32 refinement of the 24
candidates from squared coordinate differences (fp32 xyz uploaded exactly).
"""
import hashlib
import numpy as np
import concourse.bass as bass
import concourse.bacc as bacc
import concourse.tile as tile
from concourse import mybir
from concourse import bass2jax

dt = mybir.dt
AF = mybir.ActivationFunctionType
ALU = mybir.AluOpType
AX = mybir.AxisListType

N = 8192
NQ = 2048
K = 16
CHUNK = 512
NCH = N // CHUNK          # 16 chunks
NCAND = 24
NTILES = 16
BIG = 1e30
F16BIG = 60000.0
EPS = 1e-5
NEG = 0.1
GROUPS4 = [[0, 1, 2, 3], [4, 5, 6, 7]]

WN = ["LWpre", "LWq", "LWkneg", "LWv", "LWpos2a", "LWpos2b",
      "LWatt1", "LWatt2a", "LWatt2b", "LWpost"]
BN = ["Bpre", "Bv", "Battin", "Batt1", "Batt2", "Bpost", "Bpos1",
      "Gpos", "BEpos", "Gatt", "BEatt", "Gpost", "BEpost"]
WI = {n: i for i, n in enumerate(WN)}
BI_ = {n: i for i, n in enumerate(BN)}


def build(n_cores=8, ntiles=NTILES, group_size=4):
    groups = GROUPS4
    nc = bacc.Bacc("TRN2", target_bir_lowering=False, debug=False,
                   num_devices=n_cores)

    def din(name, shape, d=dt.float32):
        return nc.dram_tensor(name, shape, d, kind="ExternalInput")

    # ---- per-call data inputs (per-core slices) ----
    feat_q = din("feat_q", [128, NQ], dt.float16)
    lhsT13 = din("lhsT13", [16, NQ], dt.float16)
    rhs13s = din("rhs13s", [16, NQ], dt.float16)
    xyzsl = din("xyzsl", [4, NQ])
    pcT = din("pcT", [NQ, 4])
    # ---- weight inputs (device-cached across calls) ----
    Wall = din("Wall", [128, 128 * len(WN)])
    lhsT6 = din("lhsT6", [6, 128])
    Ball = din("Ball", [128, len(BN)])

    out = nc.dram_tensor("out", [128, NQ], dt.float32, kind="ExternalOutput")

    # internal DRAM
    nf_in = nc.dram_tensor("nf_in", [128, NQ], dt.float32)
    nf_out = nc.dram_tensor("nf_out", [group_size * 128, NQ], dt.float32)
    xr_in = nc.dram_tensor("xr_in", [4, NQ], dt.float32)
    xr_out = nc.dram_tensor("xr_out", [group_size * 4, NQ], dt.float32)
    r13_in = nc.dram_tensor("r13_in", [16, NQ], dt.float16)
    r13_out = nc.dram_tensor("r13_out", [group_size * 16, NQ], dt.float16)
    pos1_spill = nc.dram_tensor("pos1_spill", [128, ntiles * NQ], dt.float32)
    att1_spill = nc.dram_tensor("att1_spill", [128, ntiles * NQ], dt.float32)
    vg_spill = nc.dram_tensor("vg_spill", [128, ntiles * NQ], dt.float32)
    post_spill = nc.dram_tensor("post_spill", [128, ntiles * 128], dt.float32)
    cc = [(nc.dram_tensor(f"cc{i}_in", [128, 2], dt.float32),
           nc.dram_tensor(f"cc{i}_out", [128, 2], dt.float32)) for i in range(3)]

    COLS = ntiles * NQ * group_size          # N*K per batch
    M_big = 16 * COLS                        # gnorm count (pos/att)
    M_post = 16 * ntiles * 128 * group_size  # gnorm count (post)

    with tile.TileContext(nc) as tc:
        with (
            tc.tile_pool(name="pers", bufs=1) as pers,
            tc.tile_pool(name="work", bufs=1) as work,
            tc.tile_pool(name="big", bufs=4) as bigp,
            tc.tile_pool(name="psA", bufs=2, space="PSUM") as psA,
            tc.tile_pool(name="psB", bufs=2, space="PSUM") as psB,
            tc.tile_pool(name="psC", bufs=2, space="PSUM") as psC,
            tc.tile_pool(name="psT", bufs=1, space="PSUM") as psT,
        ):
            f32, f16, u16, u32, i16 = dt.float32, dt.float16, dt.uint16, dt.uint32, dt.int16

            # ---------- persistent tiles ----------
            NFQ = pers.tile([128, NQ], f32, name="NFQ")
            CT = pers.tile([4, NQ], f32, name="CT")
            LH = pers.tile([16, NQ], f16, name="LH")
            WRG = pers.tile([128, ntiles * 128], i16, name="WRG")
            WT = pers.tile([128, 128 * len(WN)], f32, name="WT")
            L16 = pers.tile([6, 128], f32, name="L16")
            L16B = pers.tile([3, 128], f32, name="L16B")
            BT = pers.tile([128, len(BN)], f32, name="BT")
            IDENT = pers.tile([128, 128], f32, name="IDENT")
            BO = pers.tile([128, 8], f32, name="BO")
            BOT = pers.tile([8, 128], f32, name="BOT")
            CB = pers.tile([128, 128], u32, name="CB")      # chunk base iota
            M0 = pers.tile([128, 384], f32, name="M0")      # refine mask
            SC384 = pers.tile([128, 384], i16, name="SC384")
            SC256 = pers.tile([128, 256], i16, name="SC256")
            STAT = pers.tile([128, 2], f32, name="STAT")    # running sums (pos)
            STAT2 = pers.tile([128, 2], f32, name="STAT2")  # (att)
            STAT3 = pers.tile([128, 2], f32, name="STAT3")  # (post)
            EPST = pers.tile([8, 1], f32, name="EPST")

            def Wap(n_):
                i = WI[n_]
                return WT[:, i * 128:(i + 1) * 128]

            def Bap(n_):
                i = BI_[n_]
                return BT[:, i:i + 1]

            # ---------- load constants ----------
            nc.sync.dma_start(WT[:], Wall.ap())
            nc.sync.dma_start(L16[:], lhsT6.ap())
            nc.sync.dma_start(L16B[:], lhsT6.ap()[3:6, :])
            nc.sync.dma_start(BT[:], Ball.ap())
            nc.sync.dma_start(LH[:], lhsT13.ap())
            nc.sync.dma_start(CT[:], xyzsl.ap())
            nc.gpsimd.memset(STAT[:], 0.0)
            nc.gpsimd.memset(STAT2[:], 0.0)
            nc.gpsimd.memset(STAT3[:], 0.0)
            nc.gpsimd.memset(EPST[:], EPS)
            nc.gpsimd.iota(CB[:], pattern=[[512, 16], [0, 8]], base=0,
                           channel_multiplier=0)

            # ---------- generate tables on device ----------
            def gent(shape, d):
                return work.tile(shape, d, tag="gen", bufs=4, name="gen")

            # IDENT[p, c] = (c == p)
            IA = gent([128, 128], u32)
            IB = gent([128, 128], u32)
            IAf = gent([128, 128], f32)
            IBf = gent([128, 128], f32)
            nc.gpsimd.iota(IA[:], pattern=[[1, 128]], base=0, channel_multiplier=0)
            nc.gpsimd.iota(IB[:], pattern=[[0, 128]], base=0, channel_multiplier=1)
            nc.vector.tensor_copy(IAf[:], IA[:])
            nc.vector.tensor_copy(IBf[:], IB[:])
            nc.vector.tensor_tensor(IDENT[:], IAf[:], IBf[:], ALU.is_equal)
            # EQ384[p, c] = (c % 16 == p % 16); M0 = EQ*BIG - BIG; SC384 = EQ*(c//16+1)-1
            A384 = gent([128, 384], u32)
            B384 = gent([128, 384], u32)
            J384 = gent([128, 384], u32)
            Af = gent([128, 384], f32)
            Bf = gent([128, 384], f32)
            Jf = gent([128, 384], f32)
            EQ = gent([128, 384], f32)
            nc.gpsimd.iota(A384[:], pattern=[[0, 24], [1, 16]], base=0,
                           channel_multiplier=0)
            nc.gpsimd.iota(B384[:], pattern=[[0, 384]], base=0, channel_multiplier=1)
            nc.gpsimd.iota(J384[:], pattern=[[1, 24], [0, 16]], base=0,
                           channel_multiplier=0)
            nc.vector.tensor_scalar(B384[:], B384[:], 15, None, ALU.bitwise_and)
            nc.vector.tensor_copy(Af[:], A384[:])
            nc.vector.tensor_copy(Bf[:], B384[:])
            nc.vector.tensor_copy(Jf[:], J384[:])
            nc.vector.tensor_tensor(EQ[:], Af[:], Bf[:], ALU.is_equal)
            nc.vector.tensor_scalar(M0[:], EQ[:], BIG, None, ALU.mult)
            nc.vector.tensor_scalar(M0[:], M0[:], BIG, None, ALU.subtract)
            nc.vector.tensor_scalar(Jf[:], Jf[:], 1.0, None, ALU.add)
            nc.vector.tensor_tensor(Jf[:], Jf[:], EQ[:], ALU.mult)
            nc.vector.tensor_scalar(Jf[:], Jf[:], 1.0, None, ALU.subtract)
            nc.vector.tensor_copy(SC384[:], Jf[:])
            nc.vector.tensor_copy(SC256[:], Jf[:, 0:256])  # same formula, 16 groups
            # BO[p, g] = (p//16 == g); BOT[g, c] = (c//16 == g)
            C8 = gent([128, 8], u32)
            G8 = gent([128, 8], u32)
            C8f = gent([128, 8], f32)
            G8f = gent([128, 8], f32)
            nc.gpsimd.iota(C8[:], pattern=[[0, 8]], base=0, channel_multiplier=1)
            nc.gpsimd.iota(G8[:], pattern=[[1, 8]], base=0, channel_multiplier=0)
            nc.vector.tensor_scalar(C8[:], C8[:], 4, None, ALU.logical_shift_right)
            nc.vector.tensor_copy(C8f[:], C8[:])
            nc.vector.tensor_copy(G8f[:], G8[:])
            nc.vector.tensor_tensor(BO[:], C8f[:], G8f[:], ALU.is_equal)
            T128 = gent([8, 128], u32)
            U128 = gent([8, 128], u32)
            T128f = gent([8, 128], f32)
            U128f = gent([8, 128], f32)
            nc.gpsimd.iota(T128[:], pattern=[[1, 128]], base=0, channel_multiplier=0)
            nc.gpsimd.iota(U128[:], pattern=[[0, 128]], base=0, channel_multiplier=1)
            nc.vector.tensor_scalar(T128[:], T128[:], 4, None, ALU.logical_shift_right)
            nc.vector.tensor_copy(T128f[:], T128[:])
            nc.vector.tensor_copy(U128f[:], U128[:])
            nc.vector.tensor_tensor(BOT[:], T128f[:], U128f[:], ALU.is_equal)

            # ---------- NFq = W_pre @ feat_q + b ----------
            for c in range(4):
                FQc = work.tile([128, 512], f16, tag="FQc", bufs=2, name="FQc")
                nc.sync.dma_start(FQc[:], feat_q.ap()[:, c * 512:(c + 1) * 512])
                FQ32 = work.tile([128, 512], f32, tag="FQ32", bufs=1, name="FQ32")
                nc.scalar.copy(FQ32[:], FQc[:])
                pb = psA.tile([128, 512], f32, tag="pA", name="pnf")
                nc.tensor.matmul(pb[:], Wap("LWpre"), FQ32[:])
                nc.scalar.activation(NFQ[:, c * 512:(c + 1) * 512], pb[:],
                                     AF.Identity, bias=Bap("Bpre"))

            # ---------- stage + collectives (xyz rows, rhs13, NF) ----------
            nc.sync.dma_start(xr_in.ap(), CT[:])
            nc.sync.dma_start(r13_in.ap(), rhs13s.ap())
            nc.sync.dma_start(nf_in.ap(), NFQ[:])
            nc.gpsimd.collective_compute(
                "AllGather", ALU.bypass, replica_groups=groups,
                ins=[xr_in.ap().opt()], outs=[xr_out.ap().opt()])
            nc.gpsimd.collective_compute(
                "AllGather", ALU.bypass, replica_groups=groups,
                ins=[r13_in.ap().opt()], outs=[r13_out.ap().opt()])
            nc.gpsimd.collective_compute(
                "AllGather", ALU.bypass, replica_groups=groups,
                ins=[nf_in.ap().opt()], outs=[nf_out.ap().opt()])

            # ================= PHASE A (per tile): KNN + pos1 =================
            # XR lives only through phase A (stack-scoped pool frees 96KB after)
            _xrp_cm = tc.tile_pool(name="xrp", bufs=1)
            xrp = _xrp_cm.__enter__()
            XR = [xrp.tile([128, N], f32, name=f"XR{c}") for c in range(3)]
            # assemble XR (replicate each coord row to 128 partitions)
            for c in range(3):
                for g in range(group_size):
                    nc.sync.dma_start(XR[c][0:1, g * NQ:(g + 1) * NQ],
                                      xr_out.ap()[g * 4 + c:g * 4 + c + 1, :])
                p = 1
                while p < 128:
                    nc.sync.dma_start(XR[c][p:2 * p, :], XR[c][0:p, :])
                    p *= 2

            for t in range(ntiles):
                toff = t * 128
                M8 = work.tile([128, 128], f16, tag="M8", name="M8")
                I8 = work.tile([128, 128], u16, tag="I8", name="I8")
                L13t = LH[:, toff:toff + 128]
                for c in range(NCH):
                    g, cg = c // 4, c % 4
                    R13c = work.tile([16, 512], f16, tag="R13c", bufs=2, name="R13c")
                    nc.sync.dma_start(
                        R13c[:],
                        r13_out.ap()[g * 16:(g + 1) * 16, cg * 512:(cg + 1) * 512])
                    pb = psA.tile([128, 512], f32, tag="pA", name="pdist")
                    nc.tensor.matmul(pb[:], L13t, R13c[:])
                    Sc = work.tile([128, 512], f16, tag="Sc", bufs=2, name="Sc")
                    nc.scalar.copy(Sc[:], pb[:])
                    nc.vector.max(M8[:, 8 * c:8 * c + 8], Sc[:])
                    nc.vector.max_index(I8[:, 8 * c:8 * c + 8],
                                        M8[:, 8 * c:8 * c + 8], Sc[:])

                # Iglob = u32(I8) + 512*chunk
                IG = work.tile([128, 128], u32, tag="IG", name="IG")
                nc.vector.tensor_copy(IG[:], I8[:])
                nc.vector.tensor_tensor(IG[:], IG[:], CB[:], ALU.add)

                # stage 2: top-24 positions of M8
                P24 = work.tile([128, 24], u16, tag="P24", name="P24")
                W8 = work.tile([128, 8], f16, tag="W8", name="W8")
                for r in range(3):
                    nc.vector.max(W8[:], M8[:])
                    nc.vector.max_index(P24[:, 8 * r:8 * r + 8], W8[:], M8[:])
                    if r < 2:
                        nc.vector.match_replace(M8[:], W8[:], M8[:], -F16BIG)

                # gather Iglob at P24 -> diag extract gidx24
                G384 = work.tile([128, 384], u32, tag="G384", name="G384")
                nc.gpsimd.ap_gather(
                    G384[:], IG[:].rearrange("p (f o) -> p f o", o=1),
                    P24[:].bitcast(i16), channels=128, num_elems=128, d=1,
                    num_idxs=384)
                G384h = work.tile([128, 384], u16, tag="G384h", name="G384h")
                nc.vector.tensor_copy(G384h[:], G384[:])
                GI24w = work.tile([128, 24], u16, tag="GI24w", name="GI24w")
                nc.gpsimd.local_scatter(GI24w[:], G384h[:], SC384[:],
                                        channels=128, num_elems=24, num_idxs=384)
                GI24 = work.tile([128, 24], u32, tag="GI24", name="GI24")
                nc.vector.tensor_copy(GI24[:], GI24w[:])

                # refine: gather xyz at candidates, exact d2
                GX = [work.tile([128, 384], f32, tag=f"GX{c}", name=f"GX{c}")
                      for c in range(3)]
                for c in range(3):
                    nc.gpsimd.ap_gather(
                        GX[c][:], XR[c][:].rearrange("p (f o) -> p f o", o=1),
                        GI24w[:].bitcast(i16), channels=128, num_elems=N, d=1,
                        num_idxs=384)
                PCt = work.tile([128, 4], f32, tag="PCt", name="PCt")
                nc.sync.dma_start(PCt[:], pcT.ap()[toff:toff + 128, :])
                SNM = work.tile([128, 384], f32, tag="SNM", name="SNM")
                SQ1 = work.tile([128, 384], f32, tag="SQS", name="SQ1")
                for c in range(3):
                    d_ = GX[c]
                    nc.vector.tensor_tensor(
                        d_[:], d_[:], PCt[:, c:c + 1].broadcast_to([128, 384]),
                        ALU.subtract)
                nc.scalar.activation(SNM[:], GX[0][:], AF.Square)
                nc.scalar.activation(SQ1[:], GX[1][:], AF.Square)
                nc.vector.tensor_tensor(SNM[:], SNM[:], SQ1[:], ALU.add)
                nc.scalar.activation(SQ1[:], GX[2][:], AF.Square)
                nc.vector.tensor_tensor(SNM[:], SNM[:], SQ1[:], ALU.add)
                # snm = M0 - d2  (own lanes: -d2; others: -BIG)
                nc.vector.tensor_tensor(SNM[:], M0[:], SNM[:], ALU.subtract)

                P16 = work.tile([128, 16], u16, tag="P16", name="P16")
                W8f = work.tile([128, 8], f32, tag="W8f", name="W8f")
                for r in range(2):
                    nc.vector.max(W8f[:], SNM[:])
                    nc.vector.max_index(P16[:, 8 * r:8 * r + 8], W8f[:], SNM[:])
                    if r < 1:
                        nc.vector.match_replace(SNM[:], W8f[:], SNM[:], -BIG)
                # c16 = P16 >> 4 (position -> candidate rank)
                C16 = work.tile([128, 16], u16, tag="C16", name="C16")
                nc.vector.tensor_scalar(C16[:], P16[:], 4, None,
                                        ALU.logical_shift_right)
                G256 = work.tile([128, 256], u32, tag="G256", name="G256")
                nc.gpsimd.ap_gather(
                    G256[:], GI24[:].rearrange("p (f o) -> p f o", o=1),
                    C16[:].bitcast(i16), channels=128, num_elems=24, d=1,
                    num_idxs=256)
                G256h = work.tile([128, 256], u16, tag="G256h", name="G256h")
                nc.vector.tensor_copy(G256h[:], G256[:])
                GI16w = work.tile([128, 16], u16, tag="GI16w", name="GI16w")
                nc.gpsimd.local_scatter(GI16w[:], G256h[:], SC256[:],
                                        channels=128, num_elems=16, num_idxs=256)
                GI16 = work.tile([128, 16], u32, tag="GI16", name="GI16")
                nc.vector.tensor_copy(GI16[:], GI16w[:])

                # wrg slot: transpose(gidx16) replicated x8
                GI16f = work.tile([128, 16], f32, tag="GI16f", name="GI16f")
                nc.vector.tensor_copy(GI16f[:], GI16[:])
                ptr = psT.tile([16, 128], f32, tag="psT", name="ptr")
                nc.tensor.transpose(ptr[:], GI16f[:], IDENT[:])
                TGf = work.tile([16, 128], f32, tag="TGf", name="TGf")
                nc.scalar.copy(TGf[:], ptr[:])
                wslot = WRG[:, t * 128:(t + 1) * 128]
                nc.vector.tensor_copy(wslot[0:16, :], TGf[:])
                p = 16
                while p < 128:
                    nc.sync.dma_start(wslot[p:2 * p, :], wslot[0:p, :])
                    p *= 2

                # pos1: split matmul (gathered neighbor xyz) - (query centers)
                PP = bigp.tile([128, NQ], f32, tag="big", name="PP")
                SQS = work.tile([128, 512], f32, tag="SQS512", name="SQS")
                A1 = work.tile([128, 1], f32, tag="A1", name="A1")
                A2 = work.tile([128, 1], f32, tag="A2", name="A2")
                for u in range(4):
                    R3 = work.tile([3, 512], f32, tag="R3", bufs=2, name="R3")
                    for c in range(3):
                        XGc = work.tile([16, 512], f32, tag="XGc", bufs=1, name="XGc")
                        nc.gpsimd.ap_gather(
                            XGc[:], XR[c][0:16, :].rearrange("p (f o) -> p f o", o=1),
                            wslot[0:16, 32 * u:32 * u + 32].bitcast(i16),
                            channels=16, num_elems=N, d=1, num_idxs=512)
                        nc.sync.dma_start(R3[c:c + 1, :], XGc[0:1, :])
                    pb = psB.tile([128, 512], f32, tag="pB", name="ppos1")
                    nc.tensor.matmul(pb[:], L16[0:3, :], R3[:],
                                     start=True, stop=False)
                    ctv = (CT[0:3, toff + 32 * u:toff + 32 * u + 32]
                           .rearrange("p (q o) -> p q o", o=1)
                           .broadcast_to([3, 32, 16]))
                    nc.tensor.matmul(pb[:], L16B[:], ctv,
                                     start=False, stop=True)
                    sl = PP[:, u * 512:(u + 1) * 512]
                    nc.scalar.activation(sl, pb[:], AF.Identity,
                                         bias=Bap("Bpos1"), accum_out=A1[:])
                    nc.scalar.activation(SQS[:], sl, AF.Square, accum_out=A2[:])
                    nc.vector.tensor_tensor(STAT[:, 0:1], STAT[:, 0:1], A1[:], ALU.add)
                    nc.vector.tensor_tensor(STAT[:, 1:2], STAT[:, 1:2], A2[:], ALU.add)
                nc.sync.dma_start(pos1_spill.ap()[:, t * NQ:(t + 1) * NQ], PP[:])

            _xrp_cm.__exit__(None, None, None)

            # ---------- allreduce stats + scale/bias ----------
            def allreduce_stats(stat, ccpair, Mcount, Gt, BEt, tag):
                ccin, ccout = ccpair
                nc.sync.dma_start(ccin.ap(), stat[:])
                nc.gpsimd.collective_compute(
                    "AllReduce", ALU.add, replica_groups=groups,
                    ins=[ccin.ap().opt()], outs=[ccout.ap().opt()])
                ST = work.tile([128, 2], f32, tag="ST" + tag, name="ST" + tag)
                nc.sync.dma_start(ST[:], ccout.ap())
                pg = psT.tile([8, 2], f32, tag="psT", name="pg" + tag)
                nc.tensor.matmul(pg[:], BO[:], ST[:])
                GS = work.tile([8, 2], f32, tag="GS" + tag, name="GS" + tag)
                nc.scalar.copy(GS[:], pg[:])
                MM = work.tile([8, 4], f32, tag="MM" + tag, name="MM" + tag)
                nc.vector.tensor_scalar(MM[:, 0:1], GS[:, 0:1], 1.0 / Mcount, None, ALU.mult)
                nc.vector.tensor_scalar(MM[:, 1:2], GS[:, 1:2], 1.0 / Mcount, None, ALU.mult)
                nc.vector.tensor_tensor(MM[:, 2:3], MM[:, 0:1], MM[:, 0:1], ALU.mult)
                nc.vector.tensor_tensor(MM[:, 2:3], MM[:, 1:2], MM[:, 2:3], ALU.subtract)
                # rs = 1/sqrt(var+eps)
                nc.scalar.activation(MM[:, 3:4], MM[:, 2:3], AF.Sqrt, bias=EPST[:])
                nc.vector.reciprocal(MM[:, 3:4], MM[:, 3:4])
                # broadcast to [128,1]
                pr = psT.tile([128, 2], f32, tag="psT", name="pr" + tag)
                nc.tensor.matmul(pr[:, 0:1], BOT[:], MM[:, 3:4])
                nc.tensor.matmul(pr[:, 1:2], BOT[:], MM[:, 0:1])
                SCB = work.tile([128, 2], f32, tag="SCB" + tag, name="SCB" + tag)
                nc.scalar.copy(SCB[:], pr[:])
                SC = work.tile([128, 1], f32, tag="SC" + tag, name="SC" + tag)
                BIt = work.tile([128, 1], f32, tag="BI" + tag, name="BI" + tag)
                nc.vector.tensor_tensor(SC[:], SCB[:, 0:1], Gt, ALU.mult)
                nc.vector.tensor_tensor(BIt[:], SCB[:, 1:2], SC[:], ALU.mult)
                nc.vector.tensor_tensor(BIt[:], BEt, BIt[:], ALU.subtract)
                return SC, BIt

            SCp, BIp = allreduce_stats(STAT, cc[0], M_big, Bap("Gpos"), Bap("BEpos"), "p")

            # ================= PHASE B (per tile) =================
            # NF (full gathered pre-conv features) lives only through phase B
            _nfp_cm = tc.tile_pool(name="nfp", bufs=1)
            nfp = _nfp_cm.__enter__()
            NF = nfp.tile([128, N], f32, name="NF")
            for g in range(group_size):
                nc.sync.dma_start(NF[:, g * NQ:(g + 1) * NQ],
                                  nf_out.ap()[g * 128:(g + 1) * 128, :])

            for t in range(ntiles):
                toff = t * 128
                PL = bigp.tile([128, NQ], f32, tag="big", name="PL")
                nc.sync.dma_start(PL[:], pos1_spill.ap()[:, t * NQ:(t + 1) * NQ])
                ZH = bigp.tile([128, NQ], f32, tag="big", name="ZH")
                ZA = bigp.tile([128, NQ], f32, tag="big", name="ZA")
                nc.scalar.activation(ZH[:], PL[:], AF.Identity, bias=BIp[:], scale=SCp[:])
                nc.scalar.activation(ZA[:], PL[:], AF.Abs, bias=BIp[:], scale=SCp[:])
                NFG = bigp.tile([128, NQ], f32, tag="big", name="NFG")
                wslot = WRG[:, t * 128:(t + 1) * 128]
                nc.gpsimd.ap_gather(
                    NFG[:], NF[:].rearrange("p (f o) -> p f o", o=1),
                    wslot.bitcast(i16), channels=128, num_elems=N, d=1, num_idxs=NQ)
                AT = bigp.tile([128, NQ], f32, tag="big", name="AT")
                A1T = bigp.tile([128, NQ], f32, tag="big", name="A1T")
                VG = bigp.tile([128, NQ], f32, tag="big", name="VG")
                SQS = work.tile([128, 512], f32, tag="SQS512", name="SQSb")
                A1 = work.tile([128, 1], f32, tag="A1", name="A1b")
                A2 = work.tile([128, 1], f32, tag="A2", name="A2b")
                for c in range(4):
                    pb = psB.tile([128, 512], f32, tag="pB", name="pattin")
                    qof = toff + c * 32
                    nc.tensor.matmul(
                        pb[:], Wap("LWq"),
                        NFQ[:, qof:qof + 32].rearrange("p (q o) -> p q o", o=1)
                        .broadcast_to([128, 32, 16]), start=True, stop=False)
                    nc.tensor.matmul(pb[:], Wap("LWkneg"),
                                     NFG[:, c * 512:(c + 1) * 512],
                                     start=False, stop=False)
                    nc.tensor.matmul(pb[:], Wap("LWpos2a"),
                                     ZH[:, c * 512:(c + 1) * 512],
                                     start=False, stop=False)
                    nc.tensor.matmul(pb[:], Wap("LWpos2b"),
                                     ZA[:, c * 512:(c + 1) * 512],
                                     start=False, stop=True)
                    nc.scalar.activation(AT[:, c * 512:(c + 1) * 512], pb[:],
                                         AF.Identity, bias=Bap("Battin"))
                    pb2 = psA.tile([128, 512], f32, tag="pA", name="patt1")
                    nc.tensor.matmul(pb2[:], Wap("LWatt1"),
                                     AT[:, c * 512:(c + 1) * 512])
                    sl = A1T[:, c * 512:(c + 1) * 512]
                    nc.scalar.activation(sl, pb2[:], AF.Identity,
                                         bias=Bap("Batt1"), accum_out=A1[:])
                    nc.scalar.activation(SQS[:], sl, AF.Square, accum_out=A2[:])
                    nc.vector.tensor_tensor(STAT2[:, 0:1], STAT2[:, 0:1], A1[:], ALU.add)
                    nc.vector.tensor_tensor(STAT2[:, 1:2], STAT2[:, 1:2], A2[:], ALU.add)
                    pb3 = psC.tile([128, 512], f32, tag="pC", name="pvg")
                    nc.tensor.matmul(pb3[:], Wap("LWv"),
                                     NFG[:, c * 512:(c + 1) * 512])
                    nc.scalar.activation(VG[:, c * 512:(c + 1) * 512], pb3[:],
                                         AF.Identity, bias=Bap("Bv"))
                nc.sync.dma_start(att1_spill.ap()[:, t * NQ:(t + 1) * NQ], A1T[:])
                nc.sync.dma_start(vg_spill.ap()[:, t * NQ:(t + 1) * NQ], VG[:])

            _nfp_cm.__exit__(None, None, None)

            SCa, BIa = allreduce_stats(STAT2, cc[1], M_big, Bap("Gatt"), Bap("BEatt"), "a")

            # ================= PHASE C (per tile) =================
            SQS128 = work.tile([128, 128], f32, tag="SQS128", name="SQS128")
            A1p = work.tile([128, 1], f32, tag="A1p", name="A1p")
            A2p = work.tile([128, 1], f32, tag="A2p", name="A2p")
            for t in range(ntiles):
                toff = t * 128
                AL = bigp.tile([128, NQ], f32, tag="big", name="AL")
                nc.sync.dma_start(AL[:], att1_spill.ap()[:, t * NQ:(t + 1) * NQ])
                AFt = bigp.tile([128, NQ], f32, tag="big", name="AFt")
                AFa = bigp.tile([128, NQ], f32, tag="big", name="AFa")
                nc.scalar.activation(AFt[:], AL[:], AF.Identity, bias=BIa[:], scale=SCa[:])
                nc.scalar.activation(AFa[:], AL[:], AF.Abs, bias=BIa[:], scale=SCa[:])
                VG = bigp.tile([128, NQ], f32, tag="big", name="VGc")
                nc.sync.dma_start(VG[:], vg_spill.ap()[:, t * NQ:(t + 1) * NQ])
                E = bigp.tile([128, NQ], f32, tag="big", name="E")
                for c in range(4):
                    pb = psB.tile([128, 512], f32, tag="pB", name="patt2")
                    nc.tensor.matmul(pb[:], Wap("LWatt2a"),
                                     AFt[:, c * 512:(c + 1) * 512],
                                     start=True, stop=False)
                    nc.tensor.matmul(pb[:], Wap("LWatt2b"),
                                     AFa[:, c * 512:(c + 1) * 512],
                                     start=False, stop=True)
                    nc.scalar.activation(E[:, c * 512:(c + 1) * 512], pb[:],
                                         AF.Exp, bias=Bap("Batt2"))
                SE = work.tile([128, 128], f32, tag="SE", name="SE")
                WS = work.tile([128, 128], f32, tag="WS", name="WS")
                EV = bigp.tile([128, NQ], f32, tag="big", name="EV")
                nc.vector.tensor_reduce(SE[:], E[:].rearrange("p (q j) -> p q j", j=16),
                                        axis=AX.X, op=ALU.add)
                nc.vector.tensor_tensor(EV[:], E[:], VG[:], ALU.mult)
                nc.vector.tensor_reduce(WS[:], EV[:].rearrange("p (q j) -> p q j", j=16),
                                        axis=AX.X, op=ALU.add)
                nc.vector.reciprocal(SE[:], SE[:])
                nc.vector.tensor_tensor(WS[:], WS[:], SE[:], ALU.mult)
                O1t = work.tile([128, 128], f32, tag="O1t", bufs=2, name="O1t")
                nc.vector.tensor_tensor(O1t[:], WS[:],
                                        NFQ[:, toff:toff + 128], ALU.add)
                # fused post conv + stats (spill to DRAM, reloaded for final norm)
                pbp = psC.tile([128, 512], f32, tag="pC", name="ppost")
                nc.tensor.matmul(pbp[:, 0:128], Wap("LWpost"), O1t[:])
                PSTc = work.tile([128, 128], f32, tag="PSTc", bufs=2, name="PSTc")
                nc.scalar.activation(PSTc[:], pbp[:, 0:128], AF.Identity,
                                     bias=Bap("Bpost"), accum_out=A1p[:])
                nc.scalar.activation(SQS128[:], PSTc[:], AF.Square, accum_out=A2p[:])
                nc.vector.tensor_tensor(STAT3[:, 0:1], STAT3[:, 0:1], A1p[:], ALU.add)
                nc.vector.tensor_tensor(STAT3[:, 1:2], STAT3[:, 1:2], A2p[:], ALU.add)
                nc.sync.dma_start(post_spill.ap()[:, toff:toff + 128], PSTc[:])

            SCq, BIq = allreduce_stats(STAT3, cc[2], M_post, Bap("Gpost"), Bap("BEpost"), "q")

            # ---------- final: leaky(norm(post)) ----------
            npost = ntiles * 128
            LD = bigp.tile([128, npost], f32, tag="big", name="LD")
            nc.sync.dma_start(LD[:], post_spill.ap())
            FZ = bigp.tile([128, npost], f32, tag="big", name="FZ")
            FA = bigp.tile([128, npost], f32, tag="big", name="FA")
            SC055 = work.tile([128, 1], f32, tag="SC055", name="SC055")
            BI055 = work.tile([128, 1], f32, tag="BI055", name="BI055")
            SC045 = work.tile([128, 1], f32, tag="SC045", name="SC045")
            BI045 = work.tile([128, 1], f32, tag="BI045", name="BI045")
            h1, h2 = (1 + NEG) / 2, (1 - NEG) / 2
            nc.vector.tensor_scalar(SC055[:], SCq[:], h1, None, ALU.mult)
            nc.vector.tensor_scalar(BI055[:], BIq[:], h1, None, ALU.mult)
            nc.vector.tensor_scalar(SC045[:], SCq[:], h2, None, ALU.mult)
            nc.vector.tensor_scalar(BI045[:], BIq[:], h2, None, ALU.mult)
            nc.scalar.activation(FZ[:], LD[:], AF.Identity, bias=BI055[:], scale=SC055[:])
            nc.scalar.activation(FA[:], LD[:], AF.Abs, bias=BI045[:], scale=SC045[:])
            nc.vector.tensor_tensor(FZ[:], FZ[:], FA[:], ALU.add)
            nc.sync.dma_start(out.ap()[:, 0:npost], FZ[:])

    nc.compile()
    return nc


# ===================== host side =====================

def _host_prep(xyz, feat):
    """Per-core per-call data inputs (global, unrotated layout)."""
    maps = []
    for b in range(2):
        xb = xyz[b].astype(np.float32)               # [3, N]
        pts = np.ascontiguousarray(xb.T)             # [N, 3]
        sq = (pts * pts).sum(-1).astype(np.float32)  # [N]
        u = (2.0 * pts).astype(np.float32)
        uhi = u.astype(np.float16)
        ulo = (u - uhi.astype(np.float32)).astype(np.float16)
        phi = pts.astype(np.float16)
        plo = (pts - phi.astype(np.float32)).astype(np.float16)
        shi = sq.astype(np.float16)
        slo = (sq - shi.astype(np.float32)).astype(np.float16)

        rhs13 = np.zeros((16, N), np.float16)
        rhs13[0:3] = phi.T
        rhs13[3:6] = plo.T
        rhs13[6:9] = phi.T
        rhs13[9] = -1.0
        rhs13[10] = -1.0
        rhs13[11] = -shi
        rhs13[12] = -slo

        lhsf = np.zeros((16, N), np.float16)
        lhsf[0:3] = uhi.T
        lhsf[3:6] = uhi.T
        lhsf[6:9] = ulo.T
        lhsf[9] = shi
        lhsf[10] = slo
        lhsf[11] = 1.0
        lhsf[12] = 1.0

        for ci in range(4):
            qs = ci * NQ
            qsl = slice(qs, qs + NQ)
            xyz_sl = np.zeros((4, NQ), np.float32)
            xyz_sl[0:3] = xb[:, qsl]
            pc = np.zeros((NQ, 4), np.float32)
            pc[:, 0:3] = pts[qsl]
            maps.append({
                "feat_q": np.ascontiguousarray(feat[b][:, qsl]).astype(np.float16),
                "lhsT13": np.ascontiguousarray(lhsf[:, qsl]),
                "rhs13s": np.ascontiguousarray(rhs13[:, qsl]),
                "xyzsl": xyz_sl,
                "pcT": pc,
            })
    return maps


def _prep_weights(W):
    lt = lambda m: np.ascontiguousarray(m.T)
    h1, h2 = (1 + NEG) / 2, (1 - NEG) / 2
    Wall = np.concatenate(
        [lt(W["W_pre"]), lt(W["W_q"]), lt(-W["W_k"]), lt(W["W_v"]),
         lt(W["W_pos2"]) * h1, lt(W["W_pos2"]) * h2, lt(W["W_att1"]),
         lt(W["W_att2"]) * h1, lt(W["W_att2"]) * h2, lt(W["W_post"])],
        axis=1).astype(np.float32)
    bcols = {
        "Bpre": W["b_pre"], "Bv": W["b_v"],
        "Battin": W["b_q"] - W["b_k"] + W["b_pos2"],
        "Batt1": W["b_att1"], "Batt2": W["b_att2"], "Bpost": W["b_post"],
        "Bpos1": W["b_pos1"], "Gpos": W["g_pos1"], "BEpos": W["be_pos1"],
        "Gatt": W["g_att1"], "BEatt": W["be_att1"],
        "Gpost": W["g_post"], "BEpost": W["be_post"],
    }
    Ball = np.stack([bcols[n].astype(np.float32) for n in BN], axis=1)
    lhsT6v = np.concatenate([W["W_pos1"].T, -W["W_pos1"].T]).astype(np.float32)
    return {"Wall": np.ascontiguousarray(Wall),
            "lhsT6": np.ascontiguousarray(lhsT6v),
            "Ball": np.ascontiguousarray(Ball)}


WEIGHT_INPUTS = ("Wall", "lhsT6", "Ball")

_CACHE = {}


def _make_runner(nc, n_cores=8):
    import jax
    from jax.sharding import Mesh, PartitionSpec
    from jax.experimental.shard_map import shard_map

    bass2jax.install_neuronx_cc_hook()
    assert nc.dbg_addr is None, "build with debug=False"
    partition_name = nc.partition_id_tensor.name if nc.partition_id_tensor else None

    in_names, out_names, out_avals = [], [], []
    for alloc in nc.m.functions[0].allocations:
        if not isinstance(alloc, mybir.MemoryLocationSet):
            continue
        name = alloc.memorylocations[0].name
        if alloc.kind == "ExternalInput":
            if name != partition_name:
                in_names.append(name)
        elif alloc.kind == "ExternalOutput":
            shape = tuple(alloc.tensor_shape)
            dtype = mybir.dt.np(alloc.dtype)
            out_names.append(name)
            out_avals.append(jax.core.ShapedArray(shape, dtype))
    n_params = len(in_names)
    n_outs = len(out_names)
    all_names = tuple(in_names + out_names + ([partition_name] if partition_name else []))
    donate = tuple(range(n_params, n_params + n_outs))

    def _body(*args):
        operands = list(args)
        if partition_name is not None:
            operands.append(bass2jax.partition_id_tensor())
        outs = bass2jax._bass_exec_p.bind(
            *operands,
            out_avals=tuple(out_avals),
            in_names=all_names,
            out_names=tuple(out_names),
            lowering_input_output_aliases=(),
            sim_require_finite=True,
            sim_require_nnan=True,
            nc=nc,
        )
        return tuple(outs)

    devices = jax.devices()[:n_cores]
    assert len(devices) == n_cores, (
        f"need {n_cores} devices, got {len(jax.devices())}")
    mesh = Mesh(np.asarray(devices), ("core",))
    in_specs = (PartitionSpec("core"),) * (n_params + n_outs)
    out_specs = (PartitionSpec("core"),) * n_outs
    fn = jax.jit(
        shard_map(_body, mesh=mesh, in_specs=in_specs, out_specs=out_specs,
                  check_rep=False),
        donate_argnums=donate, keep_unused=True)

    import jax.numpy as jnp
    from jax.sharding import NamedSharding
    zsh = tuple(NamedSharding(mesh, PartitionSpec("core")) for _ in range(n_outs))

    def _zeros():
        return tuple(jnp.zeros((n_cores * a.shape[0], *a.shape[1:]), a.dtype)
                     for a in out_avals)

    zfn = jax.jit(_zeros, out_shardings=zsh)
    return dict(fn=fn, zfn=zfn, in_names=in_names, out_names=out_names,
                out_avals=out_avals, mesh=mesh, n_cores=n_cores)


def _ensure_built():
    if "nc" not in _CACHE:
        _CACHE["nc"] = build()
        _CACHE["runner"] = _make_runner(_CACHE["nc"])
    return _CACHE["runner"]


def _run(in_maps, wmap):
    """Execute one SPMD call. in_maps: 8 per-core data dicts; wmap: weight arrays."""
    import jax
    from jax.sharding import NamedSharding, PartitionSpec
    r = _CACHE["runner"]
    n = r["n_cores"]

    h = hashlib.blake2b(digest_size=16)
    for name in WEIGHT_INPUTS:
        h.update(wmap[name].tobytes())
    wkey = h.digest()
    if _CACHE.get("wkey") != wkey:
        sh = NamedSharding(r["mesh"], PartitionSpec("core"))
        _CACHE["wdev"] = {
            name: jax.device_put(
                np.concatenate([wmap[name]] * n, axis=0), sh)
            for name in WEIGHT_INPUTS}
        _CACHE["wkey"] = wkey

    args = []
    for name in r["in_names"]:
        if name in WEIGHT_INPUTS:
            args.append(_CACHE["wdev"][name])
        else:
            args.append(np.concatenate([np.asarray(m[name]) for m in in_maps],
                                       axis=0))
    zeros = r["zfn"]()
    outs = r["fn"](*args, *zeros)
    i = r["out_names"].index("out")
    return np.asarray(outs[i]).reshape(n, *r["out_avals"][i].shape)


def kernel(**inputs) -> np.ndarray:
    xyz = np.asarray(inputs["xyz"], np.float32)    # [2, 3, 8192]
    feat = np.asarray(inputs["feat"], np.float32)  # [2, 128, 8192]
    W = {k: np.asarray(v, np.float32) for k, v in inputs.items()
         if k not in ("xyz", "feat")}

    _ensure_built()
    in_maps = _host_prep(xyz, feat)
    wmap = _prep_weights(W)
    res = _run(in_maps, wmap)

    outp = np.zeros((2, 128, N), np.float32)
    for c in range(8):
        outp[c // 4][:, (c % 4) * NQ:(c % 4 + 1) * NQ] = res[c]
    return outp
